# revision 7
# baseline (speedup 1.0000x reference)
"""DFSMN (order-9 IIR + 2-tap lookahead FIR along frames) on 8 Trainium2 cores.

Math: the reference computes, per (b, h, d) sequence along frames t:
    p[t] = base[t] + sum_{k=1..9} c_k[d] * p[t-k]
    base[t] = (1 + l0[d]) v[t] + r1[d] v[t+1] + r2[d] v[t+2]
This is a per-channel LTI filter, so p = w_d * v (convolution with the
filter's impulse response, which decays like rho^n). Each 128-frame output
block depends only on the previous ~190 input frames, which turns the whole
problem into, per channel d:

    out_block(b) = W1_d^T @ x[window b] + W2_d^T @ x[window b+1]

with W1/W2 Toeplitz matrices built on the host from the impulse response,
and windows = consecutive 128-frame chunks of the front-padded input
(conceptually 126 leading zeros + 2 trailing -> TPAD = 1152 = 9*128).

Precision: the harness gate is rel_err < 2e-2; pure-bf16 inputs with fp32
PSUM accumulation and a bf16 output give rel_err ~3e-3 (validated on host),
so no hi/lo split is needed. W1's rows 0:64 carry only lags >= 63 whose
energy is ~2e-6 of the filter, so W1 is truncated to its bottom 64 rows
and computed as a K=64 matmul. Per channel: exactly 2 matmuls
(1024 PE columns), one PSUM->SBUF bf16 copy, one in-DMA, one out-DMA.

Boundary handling, all folded into a HOST-side rank-2 correction on output
block 0 (it depends only on v[0:2]): (a) the r-tap fold pretends base
exists for t<0, (b) window 0 is all zeros except frames v[0], v[1], so the
device never loads chunk 0 at all -- its SBUF slot is memset to zero and
the v[0:2] contribution rides the same host correction.

DMA-queue discipline (three dynamic rings; HWDGE rings are FIFO per
engine, so a store that waits on compute must never sit ahead of a load
prefetch): sync ring = w2 then all x loads (minus 3 groups given to the
gpsimd SWDGE ring), scalar ring = w1 then all y stores.

Per-core tensors (host-prepared, layouts chosen for >=1KB DMA lines):
    x  [64 ch, 128 p, 8 chunk * 64 bh] bf16   p = frame-within-chunk
    w2 [128 k, 64 ch * 128 i]          bf16   one contiguous stream
    w1 [64 k,  64 ch * 128 i]          bf16   one contiguous stream
    y  [64 ch, 128 i, 8 blk * 64 bh]   bf16   i = frame-within-block
"""

import numpy as np

import concourse.bass as bass
import concourse.bacc as bacc
import concourse.mybir as mybir
from concourse import tile
from concourse import bass_utils

B, H, T, D = 16, 4, 1024, 512
N_CORES = 8
DC = D // N_CORES          # 64 channels per core
BH = B * H                 # 64 sequences (matmul free dim)
NBLK = T // 128            # 8 output blocks
NCHUNK = 9                 # SBUF chunk slots (slot 0 stays zero, 8 loaded)
F32 = mybir.dt.float32
BF16 = mybir.dt.bfloat16
FREE = NBLK * BH           # 512, matmul free dim

_NC_CACHE: dict = {}


def _build_nc(dc: int = DC):
    nc = bacc.Bacc("TRN2", target_bir_lowering=False, debug=False)
    x = nc.dram_tensor("x", [dc, 128, NBLK * BH], BF16, kind="ExternalInput")
    w2 = nc.dram_tensor("w2", [128, dc * 128], BF16, kind="ExternalInput")
    w1 = nc.dram_tensor("w1", [64, dc * 128], BF16, kind="ExternalInput")
    y = nc.dram_tensor("y", [dc, 128, FREE], BF16, kind="ExternalOutput")
    xap, yap = x.ap(), y.ap()

    XB = 4                 # channels per x DMA (524KB each)
    YB = 4                 # channels per y DMA (524KB each)
    WG = 4                 # w stream split into 4 pieces per tensor
    GP_GROUPS = (3, 7, 11)  # x groups carried by the gpsimd (SWDGE) ring

    with tile.TileContext(nc) as tc:
        with tc.tile_pool(name="xp", bufs=6) as xp, \
             tc.tile_pool(name="wp", bufs=1) as wp, \
             tc.tile_pool(name="op", bufs=4) as op, \
             tc.tile_pool(name="pp", bufs=8, space="PSUM") as pp:
            # persistent W tiles, loaded once in channel-group pieces so the
            # first matmuls only wait for their slice.  w1 lives in
            # partitions 64:128 so lhsT and rhs share a base partition in
            # the K=64 matmul (hardware requirement).
            w2t = wp.tile([128, dc * 128], BF16, name="w2t")
            w1t = wp.tile([128, dc * 128], BF16, name="w1t")
            gcols = dc * 128 // WG
            for g in range(WG):
                s2 = w2.ap().copy()
                s2.ap = s2.ap[:0] + [[dc * 128, 128], [1, gcols]]
                s2.offset = g * gcols
                nc.sync.dma_start(out=w2t[:, g * gcols:(g + 1) * gcols], in_=s2)
                s1 = w1.ap().copy()
                s1.ap = s1.ap[:0] + [[dc * 128, 64], [1, gcols]]
                s1.offset = g * gcols
                nc.scalar.dma_start(
                    out=w1t[64:128, g * gcols:(g + 1) * gcols], in_=s1)

            xt = yt = None
            for d in range(dc):
                if d % XB == 0:
                    # SBUF keeps a 9-chunk layout per channel; chunk slot 0
                    # (window 0 history) is zeroed, slots 1..8 are DMAed.
                    xt = xp.tile([128, XB * NCHUNK * BH], BF16, name="xt")
                    xvd = xt.rearrange("p (c chk bh) -> p c chk bh",
                                       c=XB, chk=NCHUNK, bh=BH)
                    nc.gpsimd.memset(xvd[64:128, :, 0, :], 0.0)
                    src = xap.copy()
                    src.ap = src.ap[:0] + [[NBLK * BH, 128],
                                           [128 * NBLK * BH, XB],
                                           [1, NBLK * BH]]
                    src.offset = d * 128 * NBLK * BH
                    eng = nc.gpsimd if (d // XB) in GP_GROUPS else nc.sync
                    eng.dma_start(out=xvd[:, :, 1:NCHUNK, :], in_=src)
                ci = d % XB
                xv = xt.rearrange("p (c chk bh) -> p c chk bh",
                                  c=XB, chk=NCHUNK, bh=BH)
                ps = pp.tile([128, FREE], F32, name="ps")
                # block b needs  W1c^T @ win_b[64:128]  +  W2^T @ win_{b+1}
                # all 8 blocks ride the free dim at once
                nc.tensor.matmul(ps, lhsT=w1t[64:128, d * 128:(d + 1) * 128],
                                 rhs=xv[64:128, ci, 0:NBLK, :],
                                 start=True, stop=False)
                nc.tensor.matmul(ps, lhsT=w2t[:, d * 128:(d + 1) * 128],
                                 rhs=xv[:, ci, 1:NBLK + 1, :],
                                 start=False, stop=True)

                if d % YB == 0:
                    yt = op.tile([128, YB * FREE], BF16, name="yt")
                ob = (d % YB) * FREE
                # PSUM -> SBUF bf16 downcast (gpsimd cannot read PSUM)
                if d % 2 == 0:
                    nc.vector.tensor_copy(yt[:, ob:ob + FREE], ps)
                else:
                    nc.scalar.copy(yt[:, ob:ob + FREE], ps)
                if d % YB == YB - 1:
                    dst = yap.copy()
                    dst.ap = dst.ap[:0] + [[FREE, 128], [128 * FREE, YB],
                                           [1, FREE]]
                    dst.offset = (d - YB + 1) * 128 * FREE
                    nc.scalar.dma_start(out=dst, in_=yt)
    nc.compile()
    return nc


def _get_nc(dc: int = DC):
    if dc not in _NC_CACHE:
        _NC_CACHE[dc] = _build_nc(dc)
    return _NC_CACHE[dc]


def _build_filters(l_filter: np.ndarray, r_filter: np.ndarray):
    """Returns w1c [64, 128, D], w2 [128, 128, D] (k, i) float64 Toeplitz
    blocks and the rank-2 block-0 boundary correction corr [2, 128, D]
    (with window-0's v[0:2] taps folded in)."""
    c = l_filter[1:].astype(np.float64)            # (9, D) IIR coeffs
    d = c.shape[1]
    a = np.zeros((258, d))
    a[0] = 1.0
    for n in range(1, 258):
        for k in range(1, min(9, n) + 1):
            a[n] += c[k - 1] * a[n - k]
    q0 = 1.0 + l_filter[0].astype(np.float64)
    q1 = r_filter[0].astype(np.float64)
    q2 = r_filter[1].astype(np.float64)

    # wseq[lag + 129] = combined FIR tap at lag, lag in [-129, 253] (0 < -2)
    wseq = np.zeros((383, d))
    for lag in range(-2, 254):
        t = q2 * a[lag + 2]
        if lag + 1 >= 0:
            t = t + q1 * a[lag + 1]
        if lag >= 0:
            t = t + q0 * a[lag]
        wseq[lag + 129] = t

    kk = np.arange(128)[:, None]
    ii = np.arange(128)[None, :]
    w1 = wseq[ii - kk + 255]                       # (128, 128, D)
    w2 = wseq[ii - kk + 127]
    # rows kk<64 of W1 only carry lags >= 63 (relative energy ~2e-6): drop
    w1c = w1[64:128]                               # (64, 128, D)

    i1 = np.arange(128)
    corr = np.stack([-(q1[None, :] * a[i1 + 1] + q2[None, :] * a[i1 + 2]),
                     -(q2[None, :] * a[i1 + 1])], axis=0)   # (2, 128, D)
    # window 0 = zeros except v[0], v[1] at w1c rows 62, 63; the device
    # skips chunk 0 entirely, so fold those two taps in here (exact f64)
    corr[0] += w1c[62]
    corr[1] += w1c[63]
    return w1c, w2, corr


def _make_in_maps(v, l_filter, r_filter, n_cores=N_CORES, dc=DC):
    import ml_dtypes
    bf16 = ml_dtypes.bfloat16
    w1c, w2, _ = _build_filters(l_filter, r_filter)

    vr = np.asarray(v, dtype=np.float32).reshape(BH, T, D)
    vpad = np.zeros((BH, T + 2, D), bf16)
    vpad[:, 0:T] = vr                              # one rounding to bf16

    w1b = w1c.astype(np.float32).astype(bf16)      # (64, 128, D)
    w2b = w2.astype(np.float32).astype(bf16)       # (128, 128, D)

    in_maps = []
    for cid in range(n_cores):
        sl = slice(cid * dc, (cid + 1) * dc)
        # x: (dc, 128 p, chunk, bh) from (bh, chunk, p, d); chunks 1..8 of
        # the padded stream = real frames [2, 1026) in blocks of 128
        xc = np.ascontiguousarray(
            vpad[:, 2:, sl].reshape(BH, NBLK, 128, dc)
            .transpose(3, 2, 1, 0)).reshape(dc, 128, NBLK * BH)
        in_maps.append({
            "x": xc,
            "w2": np.ascontiguousarray(
                w2b[:, :, sl].transpose(0, 2, 1)).reshape(128, dc * 128),
            "w1": np.ascontiguousarray(
                w1b[:, :, sl].transpose(0, 2, 1)).reshape(64, dc * 128),
        })
    return in_maps


def kernel(v: np.ndarray, l_filter: np.ndarray, r_filter: np.ndarray,
           **_unused) -> np.ndarray:
    nc = _get_nc(DC)
    in_maps = _make_in_maps(v, l_filter, r_filter)
    res = bass_utils.run_bass_kernel_spmd(nc, in_maps,
                                          core_ids=list(range(N_CORES)))

    # assemble (BH, T, D) f32 from per-core y [dc, 128 i, 8 blk, 64 bh]
    out = np.empty((BH, T, D), np.float32)
    for cid in range(N_CORES):
        yc = np.asarray(res.results[cid]["y"]).astype(np.float32)
        yc = yc.reshape(DC, 128, NBLK, BH).transpose(3, 2, 1, 0)  # bh,blk,i,d
        out[:, :, cid * DC:(cid + 1) * DC] = yc.reshape(BH, T, DC)

    # host-side rank-2 boundary correction on block 0 (depends on v[0:2])
    _, _, corr = _build_filters(l_filter, r_filter)
    vr = np.asarray(v, dtype=np.float32).reshape(BH, T, D)
    co = np.einsum("jid,bjd->bid", corr, vr[:, 0:2].astype(np.float64))
    out[:, 0:128, :] += co.astype(np.float32)
    return np.ascontiguousarray(out).reshape(B, H, T, D)


# revision 10
# speedup vs baseline: 1.2503x; 1.2503x over previous
"""DFSMN (order-9 IIR + 2-tap lookahead FIR along frames) on 8 Trainium2 cores.

Math: the reference computes, per (b, h, d) sequence along frames t:
    p[t] = base[t] + sum_{k=1..9} c_k[d] * p[t-k]
    base[t] = (1 + l0[d]) v[t] + r1[d] v[t+1] + r2[d] v[t+2]
This is a per-channel LTI filter, so p = w_d * v (convolution with the
filter's impulse response, which decays like rho^n). Each 128-frame output
block depends only on the previous ~190 input frames, which turns the whole
problem into, per channel d:

    out_block(b) = W1_d^T @ x[window b] + W2_d^T @ x[window b+1]

with W1/W2 Toeplitz matrices built on the host from the impulse response,
and windows = consecutive 128-frame chunks of the front-padded input
(conceptually 126 leading zeros + 2 trailing -> TPAD = 1152 = 9*128).

Precision: the harness gate is rel_err < 2e-2; pure-bf16 inputs with fp32
PSUM accumulation and a bf16 output give rel_err ~3e-3 (validated on host),
so no hi/lo split is needed. W1's rows 0:64 carry only lags >= 63 whose
energy is ~2e-6 of the filter, so W1 is truncated to its bottom 64 rows
and computed as a K=64 matmul. Per channel: exactly 2 matmuls
(1024 PE columns), one PSUM->SBUF bf16 copy, one in-DMA, one out-DMA.

Boundary handling, all folded into a HOST-side rank-2 correction on output
block 0 (it depends only on v[0:2]): (a) the r-tap fold pretends base
exists for t<0, (b) window 0 is all zeros except frames v[0], v[1], so the
device never loads chunk 0 at all -- its SBUF slot is memset to zero and
the v[0:2] contribution rides the same host correction.

DMA-queue discipline (three dynamic rings; HWDGE rings are FIFO per
engine, so a store that waits on compute must never sit ahead of a load
prefetch): sync ring = w2 then all x loads (minus 3 groups given to the
gpsimd SWDGE ring), scalar ring = w1 then all y stores.

Per-core tensors (host-prepared, layouts chosen for >=1KB DMA lines):
    x  [64 ch, 128 p, 8 chunk * 64 bh] bf16   p = frame-within-chunk
    w2 [128 k, 64 ch * 128 i]          bf16   one contiguous stream
    w1 [64 k,  64 ch * 128 i]          bf16   one contiguous stream
    y  [64 ch, 128 i, 8 blk * 64 bh]   bf16   i = frame-within-block
"""

import numpy as np

import concourse.bass as bass
import concourse.bacc as bacc
import concourse.mybir as mybir
from concourse import tile
from concourse import bass_utils

B, H, T, D = 16, 4, 1024, 512
N_CORES = 8
DC = D // N_CORES          # 64 channels per core
BH = B * H                 # 64 sequences (matmul free dim)
NBLK = T // 128            # 8 output blocks
NCHUNK = 9                 # SBUF chunk slots (slot 0 stays zero, 8 loaded)
F32 = mybir.dt.float32
BF16 = mybir.dt.bfloat16
FREE = NBLK * BH           # 512, matmul free dim

_NC_CACHE: dict = {}


def _build_nc(dc: int = DC):
    nc = bacc.Bacc("TRN2", target_bir_lowering=False, debug=False)
    x = nc.dram_tensor("x", [dc, 128, NBLK * BH], BF16, kind="ExternalInput")
    w2 = nc.dram_tensor("w2", [128, dc * 128], BF16, kind="ExternalInput")
    w1 = nc.dram_tensor("w1", [64, dc * 128], BF16, kind="ExternalInput")
    y = nc.dram_tensor("y", [dc, 128, FREE], BF16, kind="ExternalOutput")
    xap, yap = x.ap(), y.ap()

    XB = 4                 # channels per x DMA (524KB each)
    YB = 4                 # channels per y DMA (524KB each)
    W2G = 8                # w2 split into 8 pieces, interleaved with x loads
    W1G = 4                # w1 split into 4 pieces, front-loaded on scalar
    GP_GROUPS = (3, 7, 11)  # x groups carried by the gpsimd (SWDGE) ring
    NWU = 10               # warm-up matmuls (flip the HAM clock gate while
                           # the first x/w DMAs are still in flight)

    with tile.TileContext(nc) as tc:
        with tc.tile_pool(name="xp", bufs=8) as xp, \
             tc.tile_pool(name="wp", bufs=1) as wp, \
             tc.tile_pool(name="op", bufs=4) as op, \
             tc.tile_pool(name="wu", bufs=1, space="PSUM") as wu, \
             tc.tile_pool(name="pp", bufs=7, space="PSUM") as pp:
            # persistent W tiles.  w1 lives in partitions 64:128 so lhsT and
            # rhs share a base partition in the K=64 matmul (hw requirement).
            w2t = wp.tile([128, dc * 128], BF16, name="w2t")
            w1t = wp.tile([128, dc * 128], BF16, name="w1t")
            g1cols = dc * 128 // W1G
            for g in range(W1G):
                s1 = w1.ap().copy()
                s1.ap = s1.ap[:0] + [[dc * 128, 64], [1, g1cols]]
                s1.offset = g * g1cols
                nc.scalar.dma_start(
                    out=w1t[64:128, g * g1cols:(g + 1) * g1cols], in_=s1)

            # PE warm-up on scratch data; result never read
            wut = wp.tile([128, FREE], BF16, name="wut")
            nc.vector.memset(wut, 0.0)
            wups = wu.tile([128, FREE], F32, name="wups")
            for _ in range(NWU):
                nc.tensor.matmul(wups, lhsT=wut[:, 0:128], rhs=wut,
                                 start=True, stop=True)

            g2cols = dc * 128 // W2G
            xt = yt = None
            for d in range(dc):
                if d % XB == 0:
                    g = d // XB
                    if g % 2 == 0 and g // 2 < W2G:
                        # w2 piece k rides just ahead of x group 2k on the
                        # sync ring; it covers channel groups 2k and 2k+1
                        k = g // 2
                        s2 = w2.ap().copy()
                        s2.ap = s2.ap[:0] + [[dc * 128, 128], [1, g2cols]]
                        s2.offset = k * g2cols
                        nc.sync.dma_start(
                            out=w2t[:, k * g2cols:(k + 1) * g2cols], in_=s2)
                    xt = xp.tile([128, XB * NBLK * BH], BF16, name="xt")
                    src = xap.copy()
                    src.ap = src.ap[:0] + [[NBLK * BH, 128],
                                           [128 * NBLK * BH, XB],
                                           [1, NBLK * BH]]
                    src.offset = d * 128 * NBLK * BH
                    eng = nc.gpsimd if g in GP_GROUPS else nc.sync
                    eng.dma_start(out=xt, in_=src)
                ci = d % XB
                xv = xt.rearrange("p (c chk bh) -> p c chk bh",
                                  c=XB, chk=NBLK, bh=BH)
                ps = pp.tile([128, FREE], F32, name="ps")
                # block b needs  W2^T @ win_{b+1}  +  W1c^T @ win_b[64:128];
                # SBUF chunk slot b = window b+1.  Block 0's window-0 term
                # is zero except v[0:2], which the host folds into the
                # boundary correction, so mm1 covers blocks 1..7 only
                # (psum columns 64:512).
                nc.tensor.matmul(ps, lhsT=w2t[:, d * 128:(d + 1) * 128],
                                 rhs=xv[:, ci, 0:NBLK, :],
                                 start=True, stop=False)
                nc.tensor.matmul(ps[:, BH:FREE],
                                 lhsT=w1t[64:128, d * 128:(d + 1) * 128],
                                 rhs=xv[64:128, ci, 0:NBLK - 1, :],
                                 start=False, stop=True)

                if d % YB == 0:
                    yt = op.tile([128, YB * FREE], BF16, name="yt")
                ob = (d % YB) * FREE
                # PSUM -> SBUF bf16 downcast (gpsimd cannot read PSUM)
                if d % 2 == 0:
                    nc.vector.tensor_copy(yt[:, ob:ob + FREE], ps)
                else:
                    nc.scalar.copy(yt[:, ob:ob + FREE], ps)
                if d % YB == YB - 1:
                    dst = yap.copy()
                    dst.ap = dst.ap[:0] + [[FREE, 128], [128 * FREE, YB],
                                           [1, FREE]]
                    dst.offset = (d - YB + 1) * 128 * FREE
                    nc.scalar.dma_start(out=dst, in_=yt)
    nc.compile()
    return nc


def _get_nc(dc: int = DC):
    if dc not in _NC_CACHE:
        _NC_CACHE[dc] = _build_nc(dc)
    return _NC_CACHE[dc]


def _build_filters(l_filter: np.ndarray, r_filter: np.ndarray):
    """Returns w1c [64, 128, D], w2 [128, 128, D] (k, i) float64 Toeplitz
    blocks and the rank-2 block-0 boundary correction corr [2, 128, D]
    (with window-0's v[0:2] taps folded in)."""
    c = l_filter[1:].astype(np.float64)            # (9, D) IIR coeffs
    d = c.shape[1]
    a = np.zeros((258, d))
    a[0] = 1.0
    for n in range(1, 258):
        for k in range(1, min(9, n) + 1):
            a[n] += c[k - 1] * a[n - k]
    q0 = 1.0 + l_filter[0].astype(np.float64)
    q1 = r_filter[0].astype(np.float64)
    q2 = r_filter[1].astype(np.float64)

    # wseq[lag + 129] = combined FIR tap at lag, lag in [-129, 253] (0 < -2)
    wseq = np.zeros((383, d))
    for lag in range(-2, 254):
        t = q2 * a[lag + 2]
        if lag + 1 >= 0:
            t = t + q1 * a[lag + 1]
        if lag >= 0:
            t = t + q0 * a[lag]
        wseq[lag + 129] = t

    kk = np.arange(128)[:, None]
    ii = np.arange(128)[None, :]
    w1 = wseq[ii - kk + 255]                       # (128, 128, D)
    w2 = wseq[ii - kk + 127]
    # rows kk<64 of W1 only carry lags >= 63 (relative energy ~2e-6): drop
    w1c = w1[64:128]                               # (64, 128, D)

    i1 = np.arange(128)
    corr = np.stack([-(q1[None, :] * a[i1 + 1] + q2[None, :] * a[i1 + 2]),
                     -(q2[None, :] * a[i1 + 1])], axis=0)   # (2, 128, D)
    # window 0 = zeros except v[0], v[1] at w1c rows 62, 63; the device
    # skips chunk 0 entirely, so fold those two taps in here (exact f64)
    corr[0] += w1c[62]
    corr[1] += w1c[63]
    return w1c, w2, corr


def _make_in_maps(v, l_filter, r_filter, n_cores=N_CORES, dc=DC):
    import ml_dtypes
    bf16 = ml_dtypes.bfloat16
    w1c, w2, _ = _build_filters(l_filter, r_filter)

    vr = np.asarray(v, dtype=np.float32).reshape(BH, T, D)
    vpad = np.zeros((BH, T + 2, D), bf16)
    vpad[:, 0:T] = vr                              # one rounding to bf16

    w1b = w1c.astype(np.float32).astype(bf16)      # (64, 128, D)
    w2b = w2.astype(np.float32).astype(bf16)       # (128, 128, D)

    in_maps = []
    for cid in range(n_cores):
        sl = slice(cid * dc, (cid + 1) * dc)
        # x: (dc, 128 p, chunk, bh) from (bh, chunk, p, d); chunks 1..8 of
        # the padded stream = real frames [2, 1026) in blocks of 128
        xc = np.ascontiguousarray(
            vpad[:, 2:, sl].reshape(BH, NBLK, 128, dc)
            .transpose(3, 2, 1, 0)).reshape(dc, 128, NBLK * BH)
        in_maps.append({
            "x": xc,
            "w2": np.ascontiguousarray(
                w2b[:, :, sl].transpose(0, 2, 1)).reshape(128, dc * 128),
            "w1": np.ascontiguousarray(
                w1b[:, :, sl].transpose(0, 2, 1)).reshape(64, dc * 128),
        })
    return in_maps


def kernel(v: np.ndarray, l_filter: np.ndarray, r_filter: np.ndarray,
           **_unused) -> np.ndarray:
    nc = _get_nc(DC)
    in_maps = _make_in_maps(v, l_filter, r_filter)
    res = bass_utils.run_bass_kernel_spmd(nc, in_maps,
                                          core_ids=list(range(N_CORES)))

    # assemble (BH, T, D) f32 from per-core y [dc, 128 i, 8 blk, 64 bh]
    out = np.empty((BH, T, D), np.float32)
    for cid in range(N_CORES):
        yc = np.asarray(res.results[cid]["y"]).astype(np.float32)
        yc = yc.reshape(DC, 128, NBLK, BH).transpose(3, 2, 1, 0)  # bh,blk,i,d
        out[:, :, cid * DC:(cid + 1) * DC] = yc.reshape(BH, T, DC)

    # host-side rank-2 boundary correction on block 0 (depends on v[0:2])
    _, _, corr = _build_filters(l_filter, r_filter)
    vr = np.asarray(v, dtype=np.float32).reshape(BH, T, D)
    co = np.einsum("jid,bjd->bid", corr, vr[:, 0:2].astype(np.float64))
    out[:, 0:128, :] += co.astype(np.float32)
    return np.ascontiguousarray(out).reshape(B, H, T, D)
